# revision 28
# baseline (speedup 1.0000x reference)
"""Fused QKV projection + correlation attention (softmax over keys) on 8 trn2 cores.

Problem: x[4,2048,1024] f32; K/Q/V = x@W* + b*; out = softmax(Q Kt / 32, keys) @ V.

Weight folding: scores = Q K^T = x (Wq Wk^T) x^T, so the host precomputes
M = Wq Wk^T (pure weight preprocessing) and the device applies M once on the
key side: GT = M x_k^T.  That replaces BOTH the K and Q projections; the
score matmul consumes raw xq.  Softmax bias terms: all query-side and
constant bias terms cancel in the softmax over keys; the only survivor is
the per-key constant cb_j = x_j . (Wk bq), which the host folds (pre-scaled)
into the exp activation's per-partition bias.  bv is added on the host.

Sharding: core c -> batch b=c//2, key-half h=c%2.  Each core:
  - GT = M x_k^T for its 1024-key half; V = x_k Wv
  - scoresT[j,i] = sum_e GT[e,j] xq[i,e];  exp on ACT (scale=1/32 folded)
  - U = expT^T V (unnormalized), rs = rowsum(exp): DVE tree-sum of the 8
    key chunks + one ones[128,1] f32r matmul for the partition reduction
Host combines per-batch:  out[b] = (U0+U1)/(rs0+rs1)[:,None] + bv
(no max-subtraction needed: scores ~ N(0,1), exp stays within fp32 range).

The host permutes each core's query columns so its key half comes first:
x_k is then just xq[:, :1024] on device (one input tensor, 2MB less DMA)
and the host un-permutes U/rs rows when combining.

All matmul operands are bf16 (host pre-casts; on-chip evacs write bf16).
PE rate is 1 cycle/row for both bf16 and f32r; bf16 wins via half DMA bytes,
no f32r rounding copies, and everything resident in SBUF.  Accumulation is
fp32 in PSUM and exp runs on fp32 psum scores; measured rel err ~7e-3.

Input DMAs move full 2-4KB contiguous dram rows (chunk-granular, so the
GT-proj k-loop paces against arriving chunks); 1KB-row slices measured only
~210GB/s (descriptor-size bound) vs ~2x for full rows.  PE stream is
interleaved across query blocks so no matmul waits on an ACT/DVE evac:
warmup | GT | V | S0 S1 | rsAV0 | S2 | rsAV1 | S3 | rsAV2 | rsAV3, with the
rowsum before the last AV groups so the trailing DMAs overlap PE work, and
a ~5us warmup burst on zeros to flip the HAM clock gate during the DMA ramp.
A single 8-bank PSUM ring pool matches the emission order.
"""

import numpy as np

B, S, D = 4, 2048, 1024
N_CORES = 8

_BUILD_CACHE = {}
_RUN_KWARGS = {}      # test.py sets {"trace": True, ...} for profiling runs
_LAST_RESULTS = None  # BassKernelResults of the last run


def _build(d, sk, sq):
    """Build the per-core module. d: model dim; sk: keys/core; sq: queries/core."""
    key = (d, sk, sq)
    if key in _BUILD_CACHE:
        return _BUILD_CACHE[key]

    from contextlib import ExitStack

    import concourse.bass as bass  # noqa: F401
    import concourse.mybir as mybir
    from concourse import bacc
    from concourse.tile import TileContext

    f32 = mybir.dt.float32
    f32r = mybir.dt.float32r
    bf16 = mybir.dt.bfloat16

    P = 128
    BLK = 512                # query block / psum free-dim
    DC = d // P              # d chunks (contraction + dout chunks) = 8
    KC = sk // P             # key chunks = 8
    NBLK = sq // BLK         # query blocks = 4
    SQ4 = BLK // P           # 128-row sq chunks per block = 4
    NKB = sk // BLK          # key 512-blocks = 2
    ND = d // BLK            # d 512-blocks = 2
    scale = float(1.0 / np.sqrt(np.float32(d)))

    nc = bacc.Bacc("TRN2", target_bir_lowering=False)
    Exp = mybir.ActivationFunctionType.Exp

    xqT = nc.dram_tensor("xqT", [d, sq], bf16, kind="ExternalInput")
    M = nc.dram_tensor("M", [d, d], bf16, kind="ExternalInput")
    Wv = nc.dram_tensor("Wv", [d, d], bf16, kind="ExternalInput")
    cb = nc.dram_tensor("cb", [sk], f32, kind="ExternalInput")
    U = nc.dram_tensor("U", [sq, d], f32, kind="ExternalOutput")
    rs = nc.dram_tensor("rs", [sq], f32, kind="ExternalOutput")

    MC = 4                   # chunks per input DMA (fewer sync triggers)
    xqT_v = xqT.ap().rearrange("(g j p) s -> g p j s", j=MC, p=P)
    M_v = M.ap().rearrange("(g j p) e -> g p j e", j=MC, p=P)
    Wv_v = Wv.ap().rearrange("(g j p) e -> g p j e", j=MC, p=P)

    with TileContext(nc) as tc, ExitStack() as outer:
        resid = outer.enter_context(tc.tile_pool(name="resid", bufs=1))
        psum = outer.enter_context(tc.tile_pool(name="psum", bufs=8, space="PSUM"))
        pexp = outer.enter_context(tc.tile_pool(name="pexp", bufs=2))
        pout = outer.enter_context(tc.tile_pool(name="pout", bufs=4))
        ptree = outer.enter_context(tc.tile_pool(name="ptree", bufs=2))

        GT_sb = resid.tile([P, DC, sk], bf16)     # [d, sk]  (M-transformed keys)
        V_sb = resid.tile([P, KC, d], bf16)       # [sk, d]
        xq_sb = resid.tile([P, DC, sq], bf16)     # keys are cols [0, sk)
        M_sb = resid.tile([P, DC, d], bf16)
        Wv_sb = resid.tile([P, DC, d], bf16)
        cb_sb = resid.tile([P, KC], f32)
        ones_f = resid.tile([P, 1], f32)
        ones_b = resid.tile([P, 1], bf16)
        rs_stage = resid.tile([1, sq], f32)

        warm = resid.tile([P, BLK], bf16)
        nc.vector.memset(warm, 0.0)
        nc.vector.memset(ones_f, 1.0)
        nc.vector.tensor_copy(ones_b, ones_f)
        nc.sync.dma_start(out=cb_sb, in_=cb.ap().rearrange("(c p) -> p c", p=P))

        # ---- input DMAs: 4-chunk mega-transfers with full 2KB+ dram rows.
        # Each dma_start costs ~0.6us of sync-engine trigger time, which --
        # not bandwidth -- paced the ramp when issued per 128-row chunk.
        NG = DC // MC
        for g in range(NG):
            nc.sync.dma_start(out=M_sb[:, g * MC:(g + 1) * MC, :], in_=M_v[g])
            nc.sync.dma_start(out=xq_sb[:, g * MC:(g + 1) * MC, 0:sk],
                              in_=xqT_v[g][:, :, 0:sk])
        for g in range(NG):
            nc.sync.dma_start(out=Wv_sb[:, g * MC:(g + 1) * MC, :], in_=Wv_v[g])
            nc.sync.dma_start(out=xq_sb[:, g * MC:(g + 1) * MC, sk:sq],
                              in_=xqT_v[g][:, :, sk:sq])

        # ---- stage 0: GT and V (key half) ----------------------------------
        def gt_group(m, nb):
            # GT[dout m, key block nb] = sum_k (M^T)[k,m]^T xq[k, keys]
            ps = psum.tile([P, BLK], f32, name="ps", tag="ps")
            for k in range(DC):
                nc.tensor.matmul(
                    ps,
                    M_sb[:, k, m * P:(m + 1) * P],
                    xq_sb[:, k, nb * BLK:(nb + 1) * BLK],
                    start=(k == 0), stop=(k == DC - 1),
                )
            nc.scalar.copy(GT_sb[:, m, nb * BLK:(nb + 1) * BLK], ps)

        def vproj_group(m, nb):
            # V[key chunk m, d block nb] = sum_k xk[k,m]^T Wv[k,:] (bv on host)
            ps = psum.tile([P, BLK], f32, name="ps", tag="ps")
            for k in range(DC):
                nc.tensor.matmul(
                    ps,
                    xq_sb[:, k, m * P:(m + 1) * P],
                    Wv_sb[:, k, nb * BLK:(nb + 1) * BLK],
                    start=(k == 0), stop=(k == DC - 1),
                )
            nc.vector.tensor_copy(V_sb[:, m, nb * BLK:(nb + 1) * BLK], ps)

        # ---- stage 1 building blocks ---------------------------------------
        exp_tiles = {}
        ar_tiles = {}

        def scores(blk):
            # expT[sk, sq_blk] = exp(scale * GT^T xq + cb)
            lo = blk * BLK
            ex = pexp.tile([P, KC, BLK], bf16, name="exp")
            exp_tiles[blk] = ex
            for skc in range(KC):
                ps = psum.tile([P, BLK], f32, name="ps", tag="ps")
                for dc in range(DC):
                    nc.tensor.matmul(
                        ps, GT_sb[:, dc, skc * P:(skc + 1) * P],
                        xq_sb[:, dc, lo:lo + BLK],
                        start=(dc == 0), stop=(dc == DC - 1),
                    )
                nc.scalar.activation(
                    ex[:, skc, :], ps, Exp,
                    bias=cb_sb[:, skc:skc + 1], scale=scale,
                )
            # DVE tree-sum of the 8 key chunks; the partition reduction then
            # needs a single ones-matmul instead of 8 (frees ~6us of PE).
            # ar is bf16: an f32/f32r operand silently lowers the matmul to
            # 2-pass fp32 mode (~3x slower, seen as fp32_mode=HIGH in BIR)
            tr = ptree.tile([P, 6, BLK], f32, name="tr")
            ar = ptree.tile([P, BLK], bf16, name="ar")
            ar_tiles[blk] = ar
            for i in range(4):
                nc.vector.tensor_add(
                    tr[:, i, :], ex[:, 2 * i, :], ex[:, 2 * i + 1, :])
            nc.vector.tensor_add(tr[:, 4, :], tr[:, 0, :], tr[:, 1, :])
            nc.vector.tensor_add(tr[:, 5, :], tr[:, 2, :], tr[:, 3, :])
            nc.vector.tensor_add(ar, tr[:, 4, :], tr[:, 5, :])

        def rs_av(blk, last=False):
            lo = blk * BLK
            ex = exp_tiles.pop(blk)

            def av_group(s4, nb, row_split=1):
                sqc = blk * SQ4 + s4
                ps = psum.tile([P, BLK], f32, name="ps", tag="ps")
                for skc in range(KC):
                    nc.tensor.matmul(
                        ps, ex[:, skc, s4 * P:(s4 + 1) * P],
                        V_sb[:, skc, nb * BLK:(nb + 1) * BLK],
                        start=(skc == 0), stop=(skc == KC - 1),
                    )
                o_sb = pout.tile([P, BLK], f32, name="o_sb")
                nc.vector.tensor_copy(o_sb, ps)
                # row_split>1 fans the store over parallel queues (full 2KB
                # dram rows) so the last DMAs drain faster after the matmuls
                rp = P // row_split
                for r in range(row_split):
                    nc.sync.dma_start(
                        out=U.ap()[sqc * P + r * rp:sqc * P + (r + 1) * rp,
                                   nb * BLK:(nb + 1) * BLK],
                        in_=o_sb[r * rp:(r + 1) * rp, :],
                    )

            # AV: U[sq, d] = sum_sk expT[sk, sq]^T V[sk, d]; the row-sum
            # matmul goes before the last AV groups so its DMA (and the
            # trailing U DMAs) drain under PE work
            for s4 in range(SQ4 - 1):
                for nb in range(ND):
                    av_group(s4, nb)
            ps_rs = psum.tile([1, BLK], f32, name="ps_rs", tag="ps")
            nc.tensor.matmul(ps_rs, ones_b, ar_tiles.pop(blk),
                             start=True, stop=True)
            nc.vector.tensor_copy(rs_stage[:, lo:lo + BLK], ps_rs)
            nc.sync.dma_start(
                out=rs.ap()[lo:lo + BLK].unsqueeze(0),
                in_=rs_stage[0:1, lo:lo + BLK],
            )
            for nb in range(ND):
                av_group(SQ4 - 1, nb, row_split=2 if last else 1)

        # ---- emission order == per-engine issue order ----------------------
        # HAM warm-up: ~5us of matmuls on zeros with no DMA dependency flips
        # the PE clock gate to 8/8 while the first input chunks are landing
        for g in range(2):
            psw = psum.tile([P, BLK], f32, name="ps", tag="ps")
            for i in range(DC):
                nc.tensor.matmul(psw, warm[:, 0:P], warm,
                                 start=(i == 0), stop=(i == DC - 1))
        for nb in range(NKB):
            for m in range(DC):
                gt_group(m, nb)
        for nb in range(ND):
            for m in range(KC):
                vproj_group(m, nb)
        scores(0)
        scores(1)
        for blk in range(2, NBLK):
            rs_av(blk - 2)
            scores(blk)
        rs_av(NBLK - 2)
        rs_av(NBLK - 1)

    nc.finalize()
    _BUILD_CACHE[key] = nc
    return nc


def _numpy_fallback(x, Wk, bk, Wq, bq, Wv, bv, dims):
    k = x @ Wk + bk
    q = x @ Wq + bq
    v = x @ Wv + bv
    s = np.einsum("bqd,bkd->bqk", q, k) / np.sqrt(np.float32(q.shape[-1]))
    s = s - s.max(axis=dims, keepdims=True)
    e = np.exp(s)
    w = e / e.sum(axis=dims, keepdims=True)
    return np.einsum("bqk,bkd->bqd", w, v).astype(np.float32)


def kernel(x, Wk, bk, Wq, bq, Wv, bv, dims):
    x = np.asarray(x, np.float32)
    Wk = np.ascontiguousarray(np.asarray(Wk, np.float32))
    Wq = np.ascontiguousarray(np.asarray(Wq, np.float32))
    Wv = np.ascontiguousarray(np.asarray(Wv, np.float32))
    bk = np.ascontiguousarray(np.asarray(bk, np.float32))
    bq = np.ascontiguousarray(np.asarray(bq, np.float32))
    bv = np.ascontiguousarray(np.asarray(bv, np.float32))
    d = int(np.asarray(dims))
    if d != 2 or x.shape != (B, S, D):
        return _numpy_fallback(x, Wk, bk, Wq, bq, Wv, bv, d)

    import ml_dtypes
    from concourse.bass_utils import run_bass_kernel_spmd

    nc = _build(D, S // 2, S)

    bf = ml_dtypes.bfloat16
    cast = lambda a: np.ascontiguousarray(a.astype(bf))
    scale = np.float32(1.0 / np.sqrt(np.float32(D)))

    # weight folding (host): M = Wq Wk^T; per-key softmax bias cb = x.(Wk bq),
    # pre-scaled to match the exp activation's act(scale*psum + bias) form.
    # The device consumes M as an lhsT (computes lhsT.T @ xqT), so pass M^T.
    Ms = cast(Wk @ Wq.T)
    Wvs = cast(Wv)
    wkbq = Wk @ bq  # [D]
    half = S // 2
    in_maps = []
    xq_cache = {}
    for c in range(N_CORES):
        b, h = c // 2, c % 2
        if (b, h) not in xq_cache:
            xT16 = cast(x[b].T)  # [D, S] bf16
            # put the core's key half first: device reads keys at cols [0, half)
            if h == 0:
                xq_cache[(b, 0)] = xT16
            else:
                xq_cache[(b, 1)] = np.ascontiguousarray(
                    np.concatenate((xT16[:, half:], xT16[:, :half]), axis=1))
                xq_cache[(b, 0)] = xT16
        cb = (scale * (x[b, h * half:(h + 1) * half] @ wkbq)).astype(np.float32)
        in_maps.append({
            "xqT": xq_cache[(b, h)],
            "M": Ms, "Wv": Wvs, "cb": np.ascontiguousarray(cb),
        })

    res = run_bass_kernel_spmd(nc, in_maps, core_ids=list(range(N_CORES)),
                               **_RUN_KWARGS)
    global _LAST_RESULTS
    _LAST_RESULTS = res

    out = np.empty((B, S, D), np.float32)
    for b in range(B):
        r0, r1 = res.results[2 * b], res.results[2 * b + 1]
        u1, d1 = r1["U"], r1["rs"]
        # core h=1 worked in query-permuted order; un-permute its rows
        u1 = np.concatenate((u1[S // 2:], u1[:S // 2]), axis=0)
        d1 = np.concatenate((d1[S // 2:], d1[:S // 2]), axis=0)
        num = r0["U"] + u1
        den = r0["rs"] + d1
        out[b] = num / den[:, None] + bv
    return out


# revision 30
# speedup vs baseline: 1.0297x; 1.0297x over previous
"""Fused QKV projection + correlation attention (softmax over keys) on 8 trn2 cores.

Problem: x[4,2048,1024] f32; K/Q/V = x@W* + b*; out = softmax(Q Kt / 32, keys) @ V.

Weight folding: scores = Q K^T = x (Wq Wk^T) x^T, so the host precomputes
M = Wq Wk^T (pure weight preprocessing) and the device applies M once on the
key side: GT = M x_k^T.  That replaces BOTH the K and Q projections; the
score matmul consumes raw xq.  Softmax bias terms: all query-side and
constant bias terms cancel in the softmax over keys; the only survivor is
the per-key constant cb_j = x_j . (Wk bq), which the host folds (pre-scaled)
into the exp activation's per-partition bias.  bv is added on the host.

Sharding: core c -> batch b=c//2, key-half h=c%2.  Each core:
  - GT = M x_k^T for its 1024-key half; V = x_k Wv
  - scoresT[j,i] = sum_e GT[e,j] xq[i,e];  exp on ACT (scale=1/32 folded)
  - U = expT^T V (unnormalized), rs = rowsum(exp): DVE tree-sum of the 8
    key chunks + one ones[128,1] f32r matmul for the partition reduction
Host combines per-batch:  out[b] = (U0+U1)/(rs0+rs1)[:,None] + bv
(no max-subtraction needed: scores ~ N(0,1), exp stays within fp32 range).

The host permutes each core's query columns so its key half comes first:
x_k is then just xq[:, :1024] on device (one input tensor, 2MB less DMA)
and the host un-permutes U/rs rows when combining.

All matmul operands are bf16 (host pre-casts; on-chip evacs write bf16).
PE rate is 1 cycle/row for both bf16 and f32r; bf16 wins via half DMA bytes,
no f32r rounding copies, and everything resident in SBUF.  Accumulation is
fp32 in PSUM and exp runs on fp32 psum scores; measured rel err ~7e-3.

Input DMAs move full 2-4KB contiguous dram rows (chunk-granular, so the
GT-proj k-loop paces against arriving chunks); 1KB-row slices measured only
~210GB/s (descriptor-size bound) vs ~2x for full rows.  PE stream is
interleaved across query blocks so no matmul waits on an ACT/DVE evac:
warmup | GT | V | S0 S1 | rsAV0 | S2 | rsAV1 | S3 | rsAV2 | rsAV3, with the
rowsum before the last AV groups so the trailing DMAs overlap PE work, and
a ~5us warmup burst on zeros to flip the HAM clock gate during the DMA ramp.
A single 8-bank PSUM ring pool matches the emission order.
"""

import numpy as np

B, S, D = 4, 2048, 1024
N_CORES = 8

_BUILD_CACHE = {}
_RUN_KWARGS = {}      # test.py sets {"trace": True, ...} for profiling runs
_LAST_RESULTS = None  # BassKernelResults of the last run


def _build(d, sk, sq):
    """Build the per-core module. d: model dim; sk: keys/core; sq: queries/core."""
    key = (d, sk, sq)
    if key in _BUILD_CACHE:
        return _BUILD_CACHE[key]

    from contextlib import ExitStack

    import concourse.bass as bass  # noqa: F401
    import concourse.mybir as mybir
    from concourse import bacc
    from concourse.tile import TileContext

    f32 = mybir.dt.float32
    f32r = mybir.dt.float32r
    bf16 = mybir.dt.bfloat16

    P = 128
    BLK = 512                # query block / psum free-dim
    DC = d // P              # d chunks (contraction + dout chunks) = 8
    KC = sk // P             # key chunks = 8
    NBLK = sq // BLK         # query blocks = 4
    SQ4 = BLK // P           # 128-row sq chunks per block = 4
    NKB = sk // BLK          # key 512-blocks = 2
    ND = d // BLK            # d 512-blocks = 2
    scale = float(1.0 / np.sqrt(np.float32(d)))

    nc = bacc.Bacc("TRN2", target_bir_lowering=False)
    Exp = mybir.ActivationFunctionType.Exp

    xqT = nc.dram_tensor("xqT", [d, sq], bf16, kind="ExternalInput")
    M = nc.dram_tensor("M", [d, d], bf16, kind="ExternalInput")
    Wv = nc.dram_tensor("Wv", [d, d], bf16, kind="ExternalInput")
    cb = nc.dram_tensor("cb", [sk], f32, kind="ExternalInput")
    U = nc.dram_tensor("U", [sq, d], f32, kind="ExternalOutput")
    rs = nc.dram_tensor("rs", [sq], f32, kind="ExternalOutput")

    xqT_v = xqT.ap().rearrange("(c p) s -> c p s", p=P)
    M_v = M.ap().rearrange("(c p) e -> c p e", p=P)
    Wv_v = Wv.ap().rearrange("(c p) e -> c p e", p=P)

    with TileContext(nc) as tc, ExitStack() as outer:
        resid = outer.enter_context(tc.tile_pool(name="resid", bufs=1))
        psum = outer.enter_context(tc.tile_pool(name="psum", bufs=8, space="PSUM"))
        pexp = outer.enter_context(tc.tile_pool(name="pexp", bufs=2))
        pout = outer.enter_context(tc.tile_pool(name="pout", bufs=4))
        ptree = outer.enter_context(tc.tile_pool(name="ptree", bufs=2))

        GT_sb = resid.tile([P, DC, sk], bf16)     # [d, sk]  (M-transformed keys)
        V_sb = resid.tile([P, KC, d], bf16)       # [sk, d]
        xq_sb = resid.tile([P, DC, sq], bf16)     # keys are cols [0, sk)
        M_sb = resid.tile([P, DC, d], bf16)
        Wv_sb = resid.tile([P, DC, d], bf16)
        cb_sb = resid.tile([P, KC], f32)
        ones_f = resid.tile([P, 1], f32)
        ones_b = resid.tile([P, 1], bf16)
        rs_stage = resid.tile([1, sq], f32)

        warm = resid.tile([P, BLK], bf16)
        nc.vector.memset(warm, 0.0)
        nc.vector.memset(ones_f, 1.0)
        nc.vector.tensor_copy(ones_b, ones_f)
        nc.sync.dma_start(out=cb_sb, in_=cb.ap().rearrange("(c p) -> p c", p=P))

        # ---- input DMAs: per-chunk full contiguous dram rows, consumption
        # order.  Chunk granularity lets the GT k-loop pace against arriving
        # chunks; coarser 4-chunk transfers measured ~5us slower to first
        # wave, and 512-col slices (1KB rows) halve descriptor efficiency.
        for k in range(DC):
            nc.sync.dma_start(out=M_sb[:, k, :], in_=M_v[k])
            nc.sync.dma_start(out=xq_sb[:, k, 0:sk], in_=xqT_v[k][:, 0:sk])
        for k in range(DC):
            nc.sync.dma_start(out=Wv_sb[:, k, :], in_=Wv_v[k])
            nc.sync.dma_start(out=xq_sb[:, k, sk:sq], in_=xqT_v[k][:, sk:sq])

        # ---- stage 0: GT and V (key half) ----------------------------------
        def gt_group(m, nb):
            # GT[dout m, key block nb] = sum_k (M^T)[k,m]^T xq[k, keys]
            ps = psum.tile([P, BLK], f32, name="ps", tag="ps")
            for k in range(DC):
                nc.tensor.matmul(
                    ps,
                    M_sb[:, k, m * P:(m + 1) * P],
                    xq_sb[:, k, nb * BLK:(nb + 1) * BLK],
                    start=(k == 0), stop=(k == DC - 1),
                )
            nc.scalar.copy(GT_sb[:, m, nb * BLK:(nb + 1) * BLK], ps)

        def vproj_group(m, nb):
            # V[key chunk m, d block nb] = sum_k xk[k,m]^T Wv[k,:] (bv on host)
            ps = psum.tile([P, BLK], f32, name="ps", tag="ps")
            for k in range(DC):
                nc.tensor.matmul(
                    ps,
                    xq_sb[:, k, m * P:(m + 1) * P],
                    Wv_sb[:, k, nb * BLK:(nb + 1) * BLK],
                    start=(k == 0), stop=(k == DC - 1),
                )
            nc.vector.tensor_copy(V_sb[:, m, nb * BLK:(nb + 1) * BLK], ps)

        # ---- stage 1 building blocks ---------------------------------------
        exp_tiles = {}
        ar_tiles = {}

        def scores(blk):
            # expT[sk, sq_blk] = exp(scale * GT^T xq + cb)
            lo = blk * BLK
            ex = pexp.tile([P, KC, BLK], bf16, name="exp")
            exp_tiles[blk] = ex
            for skc in range(KC):
                ps = psum.tile([P, BLK], f32, name="ps", tag="ps")
                for dc in range(DC):
                    nc.tensor.matmul(
                        ps, GT_sb[:, dc, skc * P:(skc + 1) * P],
                        xq_sb[:, dc, lo:lo + BLK],
                        start=(dc == 0), stop=(dc == DC - 1),
                    )
                nc.scalar.activation(
                    ex[:, skc, :], ps, Exp,
                    bias=cb_sb[:, skc:skc + 1], scale=scale,
                )
            # DVE tree-sum of the 8 key chunks; the partition reduction then
            # needs a single ones-matmul instead of 8 (frees ~6us of PE).
            # ar is bf16: an f32/f32r operand silently lowers the matmul to
            # 2-pass fp32 mode (~3x slower, seen as fp32_mode=HIGH in BIR)
            tr = ptree.tile([P, 6, BLK], f32, name="tr")
            ar = ptree.tile([P, BLK], bf16, name="ar")
            ar_tiles[blk] = ar
            for i in range(4):
                nc.vector.tensor_add(
                    tr[:, i, :], ex[:, 2 * i, :], ex[:, 2 * i + 1, :])
            nc.vector.tensor_add(tr[:, 4, :], tr[:, 0, :], tr[:, 1, :])
            nc.vector.tensor_add(tr[:, 5, :], tr[:, 2, :], tr[:, 3, :])
            nc.vector.tensor_add(ar, tr[:, 4, :], tr[:, 5, :])

        def rs_av(blk, last=False):
            lo = blk * BLK
            ex = exp_tiles.pop(blk)

            def av_group(s4, nb, row_split=1):
                sqc = blk * SQ4 + s4
                ps = psum.tile([P, BLK], f32, name="ps", tag="ps")
                for skc in range(KC):
                    nc.tensor.matmul(
                        ps, ex[:, skc, s4 * P:(s4 + 1) * P],
                        V_sb[:, skc, nb * BLK:(nb + 1) * BLK],
                        start=(skc == 0), stop=(skc == KC - 1),
                    )
                o_sb = pout.tile([P, BLK], f32, name="o_sb")
                nc.vector.tensor_copy(o_sb, ps)
                # row_split>1 fans the store over parallel queues (full 2KB
                # dram rows) so the last DMAs drain faster after the matmuls
                rp = P // row_split
                for r in range(row_split):
                    nc.sync.dma_start(
                        out=U.ap()[sqc * P + r * rp:sqc * P + (r + 1) * rp,
                                   nb * BLK:(nb + 1) * BLK],
                        in_=o_sb[r * rp:(r + 1) * rp, :],
                    )

            # AV: U[sq, d] = sum_sk expT[sk, sq]^T V[sk, d]; the row-sum
            # matmul goes before the last AV groups so its DMA (and the
            # trailing U DMAs) drain under PE work
            for s4 in range(SQ4 - 1):
                for nb in range(ND):
                    av_group(s4, nb)
            ps_rs = psum.tile([1, BLK], f32, name="ps_rs", tag="ps")
            nc.tensor.matmul(ps_rs, ones_b, ar_tiles.pop(blk),
                             start=True, stop=True)
            nc.vector.tensor_copy(rs_stage[:, lo:lo + BLK], ps_rs)
            nc.sync.dma_start(
                out=rs.ap()[lo:lo + BLK].unsqueeze(0),
                in_=rs_stage[0:1, lo:lo + BLK],
            )
            for nb in range(ND):
                av_group(SQ4 - 1, nb, row_split=2 if last else 1)

        # ---- emission order == per-engine issue order ----------------------
        # HAM warm-up: ~5us of matmuls on zeros with no DMA dependency flips
        # the PE clock gate to 8/8 while the first input chunks are landing
        for g in range(2):
            psw = psum.tile([P, BLK], f32, name="ps", tag="ps")
            for i in range(DC):
                nc.tensor.matmul(psw, warm[:, 0:P], warm,
                                 start=(i == 0), stop=(i == DC - 1))
        for nb in range(NKB):
            for m in range(DC):
                gt_group(m, nb)
        for nb in range(ND):
            for m in range(KC):
                vproj_group(m, nb)
        scores(0)
        scores(1)
        for blk in range(2, NBLK):
            rs_av(blk - 2)
            scores(blk)
        rs_av(NBLK - 2)
        rs_av(NBLK - 1)

    nc.finalize()
    _BUILD_CACHE[key] = nc
    return nc


def _numpy_fallback(x, Wk, bk, Wq, bq, Wv, bv, dims):
    k = x @ Wk + bk
    q = x @ Wq + bq
    v = x @ Wv + bv
    s = np.einsum("bqd,bkd->bqk", q, k) / np.sqrt(np.float32(q.shape[-1]))
    s = s - s.max(axis=dims, keepdims=True)
    e = np.exp(s)
    w = e / e.sum(axis=dims, keepdims=True)
    return np.einsum("bqk,bkd->bqd", w, v).astype(np.float32)


def kernel(x, Wk, bk, Wq, bq, Wv, bv, dims):
    x = np.asarray(x, np.float32)
    Wk = np.ascontiguousarray(np.asarray(Wk, np.float32))
    Wq = np.ascontiguousarray(np.asarray(Wq, np.float32))
    Wv = np.ascontiguousarray(np.asarray(Wv, np.float32))
    bk = np.ascontiguousarray(np.asarray(bk, np.float32))
    bq = np.ascontiguousarray(np.asarray(bq, np.float32))
    bv = np.ascontiguousarray(np.asarray(bv, np.float32))
    d = int(np.asarray(dims))
    if d != 2 or x.shape != (B, S, D):
        return _numpy_fallback(x, Wk, bk, Wq, bq, Wv, bv, d)

    import ml_dtypes
    from concourse.bass_utils import run_bass_kernel_spmd

    nc = _build(D, S // 2, S)

    bf = ml_dtypes.bfloat16
    cast = lambda a: np.ascontiguousarray(a.astype(bf))
    scale = np.float32(1.0 / np.sqrt(np.float32(D)))

    # weight folding (host): M = Wq Wk^T; per-key softmax bias cb = x.(Wk bq),
    # pre-scaled to match the exp activation's act(scale*psum + bias) form.
    # The device consumes M as an lhsT (computes lhsT.T @ xqT), so pass M^T.
    Ms = cast(Wk @ Wq.T)
    Wvs = cast(Wv)
    wkbq = Wk @ bq  # [D]
    half = S // 2
    in_maps = []
    xq_cache = {}
    for c in range(N_CORES):
        b, h = c // 2, c % 2
        if (b, h) not in xq_cache:
            xT16 = cast(x[b].T)  # [D, S] bf16
            # put the core's key half first: device reads keys at cols [0, half)
            if h == 0:
                xq_cache[(b, 0)] = xT16
            else:
                xq_cache[(b, 1)] = np.ascontiguousarray(
                    np.concatenate((xT16[:, half:], xT16[:, :half]), axis=1))
                xq_cache[(b, 0)] = xT16
        cb = (scale * (x[b, h * half:(h + 1) * half] @ wkbq)).astype(np.float32)
        in_maps.append({
            "xqT": xq_cache[(b, h)],
            "M": Ms, "Wv": Wvs, "cb": np.ascontiguousarray(cb),
        })

    res = run_bass_kernel_spmd(nc, in_maps, core_ids=list(range(N_CORES)),
                               **_RUN_KWARGS)
    global _LAST_RESULTS
    _LAST_RESULTS = res

    out = np.empty((B, S, D), np.float32)
    for b in range(B):
        r0, r1 = res.results[2 * b], res.results[2 * b + 1]
        u1, d1 = r1["U"], r1["rs"]
        # core h=1 worked in query-permuted order; un-permute its rows
        u1 = np.concatenate((u1[S // 2:], u1[:S // 2]), axis=0)
        d1 = np.concatenate((d1[S // 2:], d1[:S // 2]), axis=0)
        num = r0["U"] + u1
        den = r0["rs"] + d1
        out[b] = num / den[:, None] + bv
    return out
